# revision 17
# baseline (speedup 1.0000x reference)
"""Cross multi-head attention kernel for 8 Trainium2 NeuronCores.

Reference computation (per batch b):
    Q = x @ Wq.T ; K = ctx @ Wk.T ; V = ctx @ Wv.T          (16 heads, depth 64)
    scores = (Q_h @ K_h.T) / 8 ; masked where pad_mask -> -inf
    att = softmax(scores) ; out_h = att @ V_h
    y = concat_h(out_h) @ fc_w.T + fc_b
Sharding: 8 cores = 2 batches x 4 head-groups (4 heads each).  Each core
computes a full [E, LQ] bf16 partial of y^T for its batch; the host sums the 4
head-group partials per batch (fp32) and adds the bias.

On-chip layout is fully transposed ("layout B") so no transposes are needed:
    x^T [E, LQ], ctx^T [E, LKV]  ->  Q^T [D,LQ], K^T [D,LKV] per head, V
    natural [LKV, D] augmented with a ones column (att@V emits softmax row
    sums for free on row 64 of the [65, LQ] accumulator).
    scores^T [LKV, LQ] = K^T.T @ Q^T       (contraction over D=64)
    att^T = exp(scores^T/8) * keep_mask^T  (exact-zero masking; no row-max
        needed: scores/8 ~ N(0,1), exp never overflows)
    y^T partial [E, LQ] = fcw_part^T.T @ attn^T   (contraction over 256)

Schedule notes (perfetto analysis of the 194us baseline):
  * Phase B was ACT-bound: 16 exps/pass (17.1us on Scalar) vs 13.6us of
    matmul -> PE starved -> HAM re-throttled the PE clock to 1.2GHz for
    ~69us of the kernel (oscillating K=4/8 windows).  Fix: spread the
    elementwise work across three engines per pass:
      - 12 tiles: ACT exp (scale=ln2/128); masks: 4 on GpSimd (bf16
        tensor_tensor, ~2.2us each, their att@V deferred to pass end),
        8 on DVE (~0.45us each).
      - 4 tiles: fused Schraudolph exp+mask on DVE in ONE op:
        K^T is pre-scaled by A16=2^7/(8 ln2) at evacuation, so the score
        PSUM holds A16*s; scalar_tensor_tensor computes
        (A16*s + B16) * keep -> int16, whose bit pattern IS bf16
        exp(s/8) (B16 = 2^7*(127-0.0354)+0.25; ~+-3.7% rel err on those
        att weights, self-normalizing via the ones-column row sums ->
        <0.2% on the output).  Masked entries multiply to int16 0 = +0.0.
  * N=1024 moving operands for scores/att@V/fc matmuls (bf16 max; halves
    instruction count vs 512).  PSUM: sc 2 banks x2 bufs + av 2x2 = 8.
  * Softmax recip via one DVE reciprocal_approx_fast (row 64 of the av
    accumulator) instead of ln+exp on ACT; broadcast over partitions with
    the zero-padded ones64 outer-product matmul as before.
  * DMA: 7+6 batched issues (was 37).  sync queue order wk, ctx(x4), wq,
    x(x2) matches the dependency chain (K gates pass 0); K pair-0 is
    pre-accumulated into a dedicated 4-bank PSUM pool (psumK) chunk-by-
    chunk as ctx quarters land, so K^T p0 completes ~1.5us after ctx.
    wv/mask(x4)/fcw are gated behind the last ctx quarter on gpsimd.
  * Pass 0 interleaves V projection + K pair-1 + its 16 score tiles in
    the DMA shadow; its att@V runs as a 16-matmul burst after psumA
    closes.  Softmax normalization of pass i runs inside pass i+1.
"""

import os
import sys

import numpy as np

for _p in ("/opt/trn_rl_repo", "/root/.axon_site/_ro/trn_rl_repo"):
    if os.path.isdir(_p) and _p not in sys.path:
        sys.path.insert(0, _p)

import ml_dtypes  # noqa: E402

import concourse.bass as bass  # noqa: E402
import concourse.mybir as mybir  # noqa: E402
import concourse.tile as tile  # noqa: E402
from concourse import bacc  # noqa: E402
from concourse.bass_utils import run_bass_kernel_spmd  # noqa: E402

B, LQ, LKV, E = 2, 1024, 2048, 1024
H_TOTAL, D = 16, 64
NCORES = 8
HGROUPS = 4          # head groups (cores per batch)
HLOCAL = 4           # heads per core
FP = HLOCAL * D      # 256 local head features
P = 128
F32 = mybir.dt.float32
BF16 = mybir.dt.bfloat16
I16 = mybir.dt.int16
ET = E // P          # 8 contraction tiles for the projections
KT = LKV // P        # 16 key tiles
PIPE = 4             # att@V runs this many kt tiles behind the scores

# Schraudolph constants: bf16 bit pattern i = 2^7*(127 + log2(e)*s/8 - delta)
A16 = 128.0 / (8.0 * float(np.log(2.0)))        # K^T pre-scale: 23.0831
B16 = 16256.0 - 128.0 * 0.0354 + 0.25           # +0.25 hedges round-vs-trunc
EXP_SCALE = float(np.log(2.0)) / 128.0          # ACT exp scale on A16*s

GP_SET = (0, 4, 8, 12)  # ACT exp, GpSimd mask       # ACT exp, GpSimd mask (att@V deferred to end)
STT_SET = (3, 7, 11, 15)   # fused DVE Schraudolph exp+mask     # fused DVE Schraudolph exp+mask
ATTV_ORDER = [kt for kt in range(KT) if kt not in GP_SET] + list(GP_SET)


def build_nc(debug: bool = False) -> bass.Bass:
    nc = bacc.Bacc("TRN2", target_bir_lowering=False)

    xT = nc.dram_tensor("xT", [E, LQ], BF16, kind="ExternalInput")
    ctxT = nc.dram_tensor("ctxT", [E, LKV], BF16, kind="ExternalInput")
    maskT = nc.dram_tensor("maskT", [LKV, LQ], BF16, kind="ExternalInput")
    wqT = nc.dram_tensor("wqT", [E, FP], BF16, kind="ExternalInput")
    wkT = nc.dram_tensor("wkT", [E, FP], BF16, kind="ExternalInput")
    wvT = nc.dram_tensor("wvT", [E, FP], BF16, kind="ExternalInput")
    fcwT = nc.dram_tensor("fcwT", [FP, E], BF16, kind="ExternalInput")
    yT = nc.dram_tensor("yT", [E, LQ], BF16, kind="ExternalOutput")
    if debug:
        dQT = nc.dram_tensor("dQT", [P, 2 * LQ], BF16, kind="ExternalOutput")
        dKT = nc.dram_tensor("dKT", [P, 2 * LKV], BF16, kind="ExternalOutput")
        dV = nc.dram_tensor("dV", [P, KT * HLOCAL * (D + 1)], BF16,
                            kind="ExternalOutput")
        dAT = nc.dram_tensor("dAT", [P, 2 * LQ], BF16, kind="ExternalOutput")

    with tile.TileContext(nc) as tc:
        with tc.tile_pool(name="persist", bufs=1) as persist:
            QT = persist.tile([P, 2, LQ], BF16)        # [:, pair, :]
            KTt = persist.tile([P, 2, LKV], BF16)      # pre-scaled by A16
            Vaug = persist.tile([P, KT, HLOCAL, P], BF16)
            attnT = persist.tile([P, 2, LQ], BF16)
            fcw_s = persist.tile([P, 2, E], BF16)
            mT_s = [
                persist.tile([P, 4, LQ], BF16, tag=f"m{j}", name=f"mT{j}")
                for j in range(4)
            ]
            # zero-padded broadcast operands: row 0 live, rows 1-127 zero so
            # the K=128 outer-product matmul is exact.
            ones64 = persist.tile([P, D], F32)
            rsr_pad = persist.tile([P, LQ], F32)

            # Preload an exp-capable table set during the DMA shadow.
            nc.scalar.add_instruction(
                mybir.InstLoadActFuncSet(
                    name=nc.scalar.bass.get_next_instruction_name(),
                    act_func_set_id=6,  # natural_log_exp_and_others
                    ins=[],
                    outs=[],
                )
            )
            nc.vector.memset(ones64[:], 0.0)
            nc.vector.memset(ones64[0:1, :], 1.0)
            nc.vector.memset(rsr_pad[:], 0.0)
            # col 0 = ones (row sums land on partition 0 of the av psum,
            # where the custom-DVE recip is valid); cols 1:64 = zeros (64-wide
            # partition reads must start 64-aligned, so V rows live at 64:128).
            nc.gpsimd.memset(Vaug[:, :, :, 0:1], 1.0)
            nc.gpsimd.memset(Vaug[:, :, :, 1:64], 0.0)

            def mask_ap(kt):
                return mT_s[kt // 4][:, kt % 4, :]

            work = None
            psumSC = None

            def emit_scores(kt, p, h):
                base = h * D
                sc = psumSC.tile([P, LQ], F32, tag="sc", bufs=2,
                                 name=f"sc_{p}{h}{kt}")
                for n in range(2):
                    nc.tensor.matmul(
                        sc[:, n * 512:(n + 1) * 512],
                        KTt[base:base + D, p, kt * P:(kt + 1) * P],
                        QT[base:base + D, p, n * 512:(n + 1) * 512],
                        start=True,
                        stop=True,
                    )
                ex = work.tile([P, LQ], BF16, tag="ex", bufs=KT,
                               name=f"ex_{p}{h}{kt}")
                if kt in STT_SET:
                    # i16 = (A16*s + B16) * keep; bit pattern == bf16 exp(s/8)
                    nc.vector.scalar_tensor_tensor(
                        ex[:].bitcast(I16),
                        sc[:],
                        B16,
                        mask_ap(kt),
                        mybir.AluOpType.add,
                        mybir.AluOpType.mult,
                    )
                else:
                    nc.scalar.activation(
                        ex[:], sc[:], mybir.ActivationFunctionType.Exp,
                        scale=EXP_SCALE,
                    )
                    eng = nc.gpsimd if kt in GP_SET else nc.vector
                    eng.tensor_tensor(
                        ex[:], ex[:], mask_ap(kt), mybir.AluOpType.mult
                    )
                return ex

            def make_norm(src, p, h, bc_pool):
                # src: [D+1, LQ] accumulator (PSUM or SBUF fp32): rows 0:D are
                # unnormalized att@V, row D the softmax row-sum.
                def emit():
                    base = h * D
                    nc.vector.reciprocal_approx_fast(
                        rsr_pad[0:1, :], src[0:1, :]
                    )
                    bc = bc_pool.tile([P, LQ], F32, tag="sc", bufs=2,
                                      name=f"bc{p}{h}")
                    for n in range(2):
                        nc.tensor.matmul(
                            bc[0:D, n * 512:(n + 1) * 512],
                            ones64[:],
                            rsr_pad[:, n * 512:(n + 1) * 512],
                            start=True,
                            stop=True,
                        )
                    bcs = work.tile([D, LQ], F32, tag="bcs", bufs=2,
                                    name=f"bcs{p}{h}")
                    nc.vector.tensor_copy(bcs[:], bc[0:D, :])
                    nc.vector.tensor_tensor(
                        attnT[base:base + D, p, :],
                        src[D:2 * D, :],
                        bcs[:],
                        mybir.AluOpType.mult,
                    )
                return emit

            with tc.tile_pool(name="work", bufs=4) as work, \
                 tc.tile_pool(name="psumSC", bufs=1, space="PSUM") as psumSC:
                # ---------------- Phase A + pass-0 scores ----------------
                ex0 = []
                with (
                    tc.tile_pool(name="inp", bufs=1) as inp,
                    tc.tile_pool(name="psumA", bufs=1, space="PSUM") as psumA,
                ):
                    wq_s = inp.tile([P, ET, FP], BF16, name="wq_s")
                    wk_s = inp.tile([P, ET, FP], BF16, name="wk_s")
                    wv_s = inp.tile([P, ET, FP], BF16, name="wv_s")
                    xT_s = [inp.tile([P, 4, LQ], BF16, tag=f"xT{j}",
                                     name=f"xT{j}") for j in range(2)]
                    cT_s = [inp.tile([P, 2, LKV], BF16, tag=f"cT{j}",
                                     name=f"cT{j}") for j in range(4)]

                    def x_chunk(k):
                        return xT_s[k // 4][:, k % 4, :]

                    def c_chunk(k):
                        return cT_s[k // 2][:, k % 2, :]

                    # Primary input stream, one queue (sync), dependency
                    # order: K inputs first (K gates pass 0), then Q's.
                    nc.sync.dma_start(
                        wk_s[:], wkT.rearrange("(ko pi) f -> pi ko f", pi=P)
                    )
                    for j in range(4):
                        nc.sync.dma_start(
                            cT_s[j][:],
                            ctxT[j * 256:(j + 1) * 256, :].rearrange(
                                "(ko pi) k -> pi ko k", pi=P
                            ),
                        )
                    nc.sync.dma_start(
                        wq_s[:], wqT.rearrange("(ko pi) f -> pi ko f", pi=P)
                    )
                    for j in range(2):
                        nc.sync.dma_start(
                            xT_s[j][:],
                            xT[j * 512:(j + 1) * 512, :].rearrange(
                                "(ko pi) q -> pi ko q", pi=P
                            ),
                        )

                    # Second stream gated behind the last ctx quarter so it
                    # doesn't steal HBM bandwidth from the critical path.
                    gate = inp.tile([1, 1], BF16, name="gate")
                    nc.gpsimd.tensor_copy(gate[:], cT_s[3][0:1, 1, 0:1])
                    nc.gpsimd.dma_start(
                        wv_s[:], wvT.rearrange("(ko pi) f -> pi ko f", pi=P)
                    )
                    for j in range(4):
                        nc.gpsimd.dma_start(
                            mT_s[j][:],
                            maskT[j * 512:(j + 1) * 512, :].rearrange(
                                "(kt pi) q -> pi kt q", pi=P
                            ),
                        )
                    nc.gpsimd.dma_start(
                        fcw_s[:], fcwT.rearrange("(ko pi) e -> pi ko e", pi=P)
                    )

                    # K pair 0, chunk-major across two [P, LQ] tiles borrowed
                    # from the sc tag (psumSC is otherwise idle in phase A):
                    # all output tiles accumulate chunk-by-chunk as ctx lands.
                    kp0 = [
                        psumSC.tile([P, LQ], F32, tag="sc", bufs=2,
                                    name=f"kp0_{n}")
                        for n in range(2)
                    ]
                    for k in range(ET):
                        for n in range(4):
                            nc.tensor.matmul(
                                kp0[n // 2][:, (n % 2) * 512:(n % 2 + 1) * 512],
                                wk_s[:, k, 0:P],
                                c_chunk(k)[:, n * 512:(n + 1) * 512],
                                start=(k == 0),
                                stop=(k == ET - 1),
                            )
                    for n in range(2):
                        nc.scalar.mul(
                            KTt[:, 0, n * 1024:(n + 1) * 1024], kp0[n][:], A16
                        )

                    # Q^T, both pairs, chunk-major (needs only x + wq).
                    qp = [
                        psumA.tile([P, LQ], F32, tag="ps1k", bufs=2,
                                   name=f"qp_{p}")
                        for p in range(2)
                    ]
                    for k in range(ET):
                        for p in range(2):
                            for n in range(2):
                                nc.tensor.matmul(
                                    qp[p][:, n * 512:(n + 1) * 512],
                                    wq_s[:, k, p * P:(p + 1) * P],
                                    x_chunk(k)[:, n * 512:(n + 1) * 512],
                                    start=(k == 0),
                                    stop=(k == ET - 1),
                                )
                    for p in range(2):
                        nc.vector.tensor_copy(QT[:, p, :], qp[p][:])

                    # Pass-0 scores + V projection + K pair 1, interleaved.
                    # V packs 4 LKV tiles side by side into one [P, LQ] psum.
                    def k_p1(n):
                        ps = psumA.tile([P, LQ], F32, tag="ps1k", bufs=2)
                        for k in range(ET):
                            for m in range(2):
                                nc.tensor.matmul(
                                    ps[:, m * 512:(m + 1) * 512],
                                    wk_s[:, k, P:2 * P],
                                    c_chunk(k)[:, n * 1024 + m * 512:
                                               n * 1024 + (m + 1) * 512],
                                    start=(k == 0),
                                    stop=(k == ET - 1),
                                )
                        nc.scalar.mul(
                            KTt[:, 1, n * 1024:(n + 1) * 1024], ps[:], A16
                        )

                    for g in range(4):
                        vt = psumA.tile([P, LQ], F32, tag="ps1k", bufs=2,
                                        name=f"vt{g}")
                        for j in range(4):
                            mv = 4 * g + j
                            ex0.append(emit_scores(mv, 0, 0))
                            for k in range(ET):
                                nc.tensor.matmul(
                                    vt[:, j * FP:(j + 1) * FP],
                                    c_chunk(k)[:, mv * P:(mv + 1) * P],
                                    wv_s[:, k, :],
                                    start=(k == 0),
                                    stop=(k == ET - 1),
                                )
                            nc.vector.tensor_copy(
                                Vaug[:, mv, :, D:2 * D],
                                vt[:, j * FP:(j + 1) * FP].rearrange(
                                    "p (h d) -> p h d", d=D
                                ),
                            )
                        if g % 2 == 1:
                            k_p1(g // 2)

                # ---------------- Phase B: attention ----------------
                norm_pending = None
                av3_s = None
                with tc.tile_pool(name="psumAV", bufs=1,
                                  space="PSUM") as psumAV:
                    for pi in range(4):
                        p, h = divmod(pi, 2)
                        hh = 2 * p + h
                        av = psumAV.tile([P, LQ], F32, tag="av", bufs=2,
                                         name=f"av{hh}")

                        def attv(okt, oex, pos, av=av, hh=hh):
                            for n in range(2):
                                nc.tensor.matmul(
                                    av[:, n * 512:(n + 1) * 512],
                                    Vaug[:, okt, hh, :],
                                    oex[:, n * 512:(n + 1) * 512],
                                    start=(pos == 0),
                                    stop=(pos == KT - 1),
                                )

                        if pi == 0:
                            for kt in range(KT):
                                attv(kt, ex0[kt], kt)
                        else:
                            exs = {}
                            pos = 0
                            for i, kt in enumerate(range(KT)):
                                exs[kt] = emit_scores(kt, p, h)
                                if i == 3 and norm_pending is not None:
                                    norm_pending()
                                    norm_pending = None
                                if i >= PIPE:
                                    okt = ATTV_ORDER[i - PIPE]
                                    attv(okt, exs[okt], pos)
                                    pos += 1
                            for j in range(KT - PIPE, KT):
                                okt = ATTV_ORDER[j]
                                attv(okt, exs[okt], pos)
                                pos += 1

                        if pi < 3:
                            norm_pending = make_norm(av, p, h, psumSC)
                        else:
                            # normalize in place while av is still PSUM (a
                            # tensor_tensor with both inputs in SBUF would
                            # need equal base partitions).
                            make_norm(av, p, h, psumSC)()

                # ---------------- Phase C: output projection ----------------
                with (
                    tc.tile_pool(name="psumC", bufs=1, space="PSUM") as psumC,
                    tc.tile_pool(name="outp", bufs=4) as outp,
                ):
                    CCH = 8
                    ps_c = [None] * CCH

                    def fc_mm(c, kf):
                        if kf == 0:
                            ps_c[c] = psumC.tile([P, LQ], F32, tag="fc",
                                                 bufs=2, name=f"fc{c}")
                        for n in range(2):
                            nc.tensor.matmul(
                                ps_c[c][:, n * 512:(n + 1) * 512],
                                fcw_s[:, kf, c * P:(c + 1) * P],
                                attnT[:, kf, n * 512:(n + 1) * 512],
                                start=(kf == 0),
                                stop=(kf == 1),
                            )

                    for c in range(2):
                        fc_mm(c, 0)
                    for c in range(CCH):
                        fc_mm(c, 1)
                        ob = outp.tile([P, LQ], BF16, tag="ob", bufs=4,
                                       name=f"ob{c}")
                        if c % 2 == 0:
                            nc.scalar.copy(ob[:], ps_c[c][:])
                        else:
                            nc.vector.tensor_copy(ob[:], ps_c[c][:])
                        eng = nc.sync if c % 2 == 0 else nc.gpsimd
                        eng.dma_start(yT[c * P:(c + 1) * P, :], ob[:])
                        if c + 2 < CCH:
                            fc_mm(c + 2, 0)
                    if debug:
                        nc.sync.dma_start(
                            dQT[:, :], QT.rearrange("p a q -> p (a q)"))
                        nc.sync.dma_start(
                            dKT[:, :], KTt.rearrange("p a q -> p (a q)"))
                        nc.sync.dma_start(
                            dV[:, :], Vaug.rearrange("p a b c -> p (a b c)"))
                        nc.sync.dma_start(
                            dAT[:, :], attnT.rearrange("p a q -> p (a q)"))

    nc.compile()
    return nc


_NC_CACHE: dict = {}


def _get_nc() -> bass.Bass:
    if "nc" not in _NC_CACHE:
        _NC_CACHE["nc"] = build_nc()
    return _NC_CACHE["nc"]


def make_in_maps(x, context, pad_mask, Wq, Wk, Wv, fc_w):
    x = np.asarray(x, dtype=np.float32)
    context = np.asarray(context, dtype=np.float32)
    pad_mask = np.asarray(pad_mask).astype(bool)
    Wq = np.asarray(Wq, dtype=np.float32)
    Wk = np.asarray(Wk, dtype=np.float32)
    Wv = np.asarray(Wv, dtype=np.float32)
    fc_w = np.asarray(fc_w, dtype=np.float32)

    xT = np.ascontiguousarray(x.transpose(0, 2, 1)).astype(ml_dtypes.bfloat16)
    cT = np.ascontiguousarray(context.transpose(0, 2, 1)).astype(ml_dtypes.bfloat16)
    keepT = np.ascontiguousarray(
        (~pad_mask).transpose(0, 2, 1)
    ).astype(ml_dtypes.bfloat16)                                    # [B, LKV, LQ]

    in_maps = []
    for c in range(NCORES):
        b, hg = divmod(c, HGROUPS)
        fsl = slice(hg * FP, (hg + 1) * FP)
        in_maps.append(
            {
                "xT": xT[b],
                "ctxT": cT[b],
                "maskT": keepT[b],
                "wqT": np.ascontiguousarray(Wq[fsl, :].T).astype(ml_dtypes.bfloat16),
                "wkT": np.ascontiguousarray(Wk[fsl, :].T).astype(ml_dtypes.bfloat16),
                "wvT": np.ascontiguousarray(Wv[fsl, :].T).astype(ml_dtypes.bfloat16),
                "fcwT": np.ascontiguousarray(fc_w[:, fsl].T).astype(ml_dtypes.bfloat16),
            }
        )
    return in_maps


def _combine(outs, fc_b):
    fc_b = np.asarray(fc_b, dtype=np.float32)
    y = np.empty((B, LQ, E), dtype=np.float32)
    for b in range(B):
        acc = outs[HGROUPS * b].astype(np.float32)
        for g in range(1, HGROUPS):
            acc = acc + outs[HGROUPS * b + g].astype(np.float32)
        y[b] = acc.T + fc_b
    return y


def run_traced(x, context, pad_mask, Wq, Wk, Wv, fc_w, fc_b, trace=False):
    nc = _get_nc()
    in_maps = make_in_maps(x, context, pad_mask, Wq, Wk, Wv, fc_w)
    res = run_bass_kernel_spmd(nc, in_maps, list(range(NCORES)), trace=trace)
    outs = [r["yT"] for r in res.results]
    return _combine(outs, fc_b), res


def kernel(x, context, pad_mask, Wq, Wk, Wv, fc_w, fc_b):
    y, _ = run_traced(x, context, pad_mask, Wq, Wk, Wv, fc_w, fc_b, trace=False)
    return y


# revision 19
# speedup vs baseline: 1.0716x; 1.0716x over previous
"""Cross multi-head attention kernel for 8 Trainium2 NeuronCores.

Reference computation (per batch b):
    Q = x @ Wq.T ; K = ctx @ Wk.T ; V = ctx @ Wv.T          (16 heads, depth 64)
    scores = (Q_h @ K_h.T) / 8 ; masked where pad_mask -> -inf
    att = softmax(scores) ; out_h = att @ V_h
    y = concat_h(out_h) @ fc_w.T + fc_b
Sharding: 8 cores = 2 batches x 4 head-groups (4 heads each).  Each core
computes a full [E, LQ] bf16 partial of y^T for its batch; the host sums the 4
head-group partials per batch (fp32) and adds the bias.

On-chip layout is fully transposed ("layout B") so no transposes are needed:
    x^T [E, LQ], ctx^T [E, LKV]  ->  Q^T [D,LQ], K^T [D,LKV] per head, V
    natural [LKV, D] augmented with a ones column (att@V emits softmax row
    sums for free on row 64 of the [65, LQ] accumulator).
    scores^T [LKV, LQ] = K^T.T @ Q^T       (contraction over D=64)
    att^T = exp(scores^T/8) * keep_mask^T  (exact-zero masking; no row-max
        needed: scores/8 ~ N(0,1), exp never overflows)
    y^T partial [E, LQ] = fcw_part^T.T @ attn^T   (contraction over 256)

Schedule notes (perfetto analysis of the 194us baseline):
  * Phase B was ACT-bound: 16 exps/pass (17.1us on Scalar) vs 13.6us of
    matmul -> PE starved -> HAM re-throttled the PE clock to 1.2GHz for
    ~69us of the kernel (oscillating K=4/8 windows).  Fix: spread the
    elementwise work across three engines per pass:
      - 12 tiles: ACT exp (scale=ln2/128); masks: 4 on GpSimd (bf16
        tensor_tensor, ~2.2us each, their att@V deferred to pass end),
        8 on DVE (~0.45us each).
      - 4 tiles: fused Schraudolph exp+mask on DVE in ONE op:
        K^T is pre-scaled by A16=2^7/(8 ln2) at evacuation, so the score
        PSUM holds A16*s; scalar_tensor_tensor computes
        (A16*s + B16) * keep -> int16, whose bit pattern IS bf16
        exp(s/8) (B16 = 2^7*(127-0.0354)+0.25; ~+-3.7% rel err on those
        att weights, self-normalizing via the ones-column row sums ->
        <0.2% on the output).  Masked entries multiply to int16 0 = +0.0.
  * N=1024 moving operands for scores/att@V/fc matmuls (bf16 max; halves
    instruction count vs 512).  PSUM: sc 2 banks x2 bufs + av 2x2 = 8.
  * Softmax recip via one DVE reciprocal_approx_fast (row 64 of the av
    accumulator) instead of ln+exp on ACT; broadcast over partitions with
    the zero-padded ones64 outer-product matmul as before.
  * DMA: 7+6 batched issues (was 37).  sync queue order wk, ctx(x4), wq,
    x(x2) matches the dependency chain (K gates pass 0); K pair-0 is
    pre-accumulated into a dedicated 4-bank PSUM pool (psumK) chunk-by-
    chunk as ctx quarters land, so K^T p0 completes ~1.5us after ctx.
    wv/mask(x4)/fcw are gated behind the last ctx quarter on gpsimd.
  * Pass 0 interleaves V projection + K pair-1 + its 16 score tiles in
    the DMA shadow; its att@V runs as a 16-matmul burst after psumA
    closes.  Softmax normalization of pass i runs inside pass i+1.
"""

import os
import sys

import numpy as np

for _p in ("/opt/trn_rl_repo", "/root/.axon_site/_ro/trn_rl_repo"):
    if os.path.isdir(_p) and _p not in sys.path:
        sys.path.insert(0, _p)

import ml_dtypes  # noqa: E402

import concourse.bass as bass  # noqa: E402
import concourse.mybir as mybir  # noqa: E402
import concourse.tile as tile  # noqa: E402
from concourse import bacc  # noqa: E402
from concourse.bass_utils import run_bass_kernel_spmd  # noqa: E402

B, LQ, LKV, E = 2, 1024, 2048, 1024
H_TOTAL, D = 16, 64
NCORES = 8
HGROUPS = 4          # head groups (cores per batch)
HLOCAL = 4           # heads per core
FP = HLOCAL * D      # 256 local head features
P = 128
F32 = mybir.dt.float32
BF16 = mybir.dt.bfloat16
I16 = mybir.dt.int16
ET = E // P          # 8 contraction tiles for the projections
KT = LKV // P        # 16 key tiles
PIPE = 6             # att@V runs this many kt tiles behind the scores

# Schraudolph constants: bf16 bit pattern i = 2^7*(127 + log2(e)*s/8 - delta)
A16 = 128.0 / (8.0 * float(np.log(2.0)))        # K^T pre-scale: 23.0831
B16 = 16256.0 - 128.0 * 0.0354 + 0.25           # +0.25 hedges round-vs-trunc
EXP_SCALE = float(np.log(2.0)) / 128.0          # ACT exp scale on A16*s

GP_SET = (0, 4, 8, 12)  # ACT exp, GpSimd mask       # ACT exp, GpSimd mask (att@V deferred to end)
STT_SET = (3, 7, 11, 15)   # fused DVE Schraudolph exp+mask     # fused DVE Schraudolph exp+mask
ATTV_ORDER = [kt for kt in range(KT) if kt not in GP_SET] + list(GP_SET)


def build_nc(debug: bool = False) -> bass.Bass:
    nc = bacc.Bacc("TRN2", target_bir_lowering=False)

    xT = nc.dram_tensor("xT", [E, LQ], BF16, kind="ExternalInput")
    ctxT = nc.dram_tensor("ctxT", [E, LKV], BF16, kind="ExternalInput")
    maskT = nc.dram_tensor("maskT", [LKV, LQ], BF16, kind="ExternalInput")
    wqT = nc.dram_tensor("wqT", [E, FP], BF16, kind="ExternalInput")
    wkT = nc.dram_tensor("wkT", [E, FP], BF16, kind="ExternalInput")
    wvT = nc.dram_tensor("wvT", [E, FP], BF16, kind="ExternalInput")
    fcwT = nc.dram_tensor("fcwT", [FP, E], BF16, kind="ExternalInput")
    yT = nc.dram_tensor("yT", [E, LQ], BF16, kind="ExternalOutput")
    if debug:
        dQT = nc.dram_tensor("dQT", [P, 2 * LQ], BF16, kind="ExternalOutput")
        dKT = nc.dram_tensor("dKT", [P, 2 * LKV], BF16, kind="ExternalOutput")
        dV = nc.dram_tensor("dV", [P, KT * HLOCAL * (D + 1)], BF16,
                            kind="ExternalOutput")
        dAT = nc.dram_tensor("dAT", [P, 2 * LQ], BF16, kind="ExternalOutput")

    with tile.TileContext(nc) as tc:
        with tc.tile_pool(name="persist", bufs=1) as persist:
            QT = persist.tile([P, 2, LQ], BF16)        # [:, pair, :]
            KTt = persist.tile([P, 2, LKV], BF16)      # pre-scaled by A16
            Vaug = persist.tile([P, KT, HLOCAL, P], BF16)
            attnT = persist.tile([P, 2, LQ], BF16)
            fcw_s = persist.tile([P, 2, E], BF16)
            mT_all = persist.tile([P, KT * LQ], BF16, name="mT_all")
            # zero-padded broadcast operands: row 0 live, rows 1-127 zero so
            # the K=128 outer-product matmul is exact.
            ones64 = persist.tile([P, D], F32)
            rsr_pad = persist.tile([P, LQ], F32)

            # Preload an exp-capable table set during the DMA shadow.
            nc.scalar.add_instruction(
                mybir.InstLoadActFuncSet(
                    name=nc.scalar.bass.get_next_instruction_name(),
                    act_func_set_id=6,  # natural_log_exp_and_others
                    ins=[],
                    outs=[],
                )
            )
            nc.vector.memset(ones64[:], 0.0)
            nc.vector.memset(ones64[0:1, :], 1.0)
            nc.vector.memset(rsr_pad[:], 0.0)
            # col 0 = ones (row sums land on partition 0 of the av psum,
            # where the custom-DVE recip is valid); cols 1:64 = zeros (64-wide
            # partition reads must start 64-aligned, so V rows live at 64:128).
            nc.gpsimd.memset(Vaug[:, :, :, 0:1], 1.0)
            nc.gpsimd.memset(Vaug[:, :, :, 1:64], 0.0)

            def mask_ap(kt):
                return mT_all[:, kt * LQ:(kt + 1) * LQ]

            work = None
            psumSC = None

            def emit_scores(kt, p, h):
                base = h * D
                sc = psumSC.tile([P, LQ], F32, tag="sc", bufs=2,
                                 name=f"sc_{p}{h}{kt}")
                for n in range(2):
                    nc.tensor.matmul(
                        sc[:, n * 512:(n + 1) * 512],
                        KTt[base:base + D, p, kt * P:(kt + 1) * P],
                        QT[base:base + D, p, n * 512:(n + 1) * 512],
                        start=True,
                        stop=True,
                    )
                ex = work.tile([P, LQ], BF16, tag="ex", bufs=KT,
                               name=f"ex_{p}{h}{kt}")
                if kt in STT_SET:
                    # i16 = (A16*s + B16) * keep; bit pattern == bf16 exp(s/8)
                    nc.vector.scalar_tensor_tensor(
                        ex[:].bitcast(I16),
                        sc[:],
                        B16,
                        mask_ap(kt),
                        mybir.AluOpType.add,
                        mybir.AluOpType.mult,
                    )
                else:
                    nc.scalar.activation(
                        ex[:], sc[:], mybir.ActivationFunctionType.Exp,
                        scale=EXP_SCALE,
                    )
                    eng = nc.gpsimd if kt in GP_SET else nc.vector
                    eng.tensor_tensor(
                        ex[:], ex[:], mask_ap(kt), mybir.AluOpType.mult
                    )
                return ex

            def make_norm(src, p, h, bc_pool):
                # src: [D+1, LQ] accumulator (PSUM or SBUF fp32): rows 0:D are
                # unnormalized att@V, row D the softmax row-sum.
                def emit():
                    base = h * D
                    nc.vector.reciprocal_approx_fast(
                        rsr_pad[0:1, :], src[0:1, :]
                    )
                    bc = bc_pool.tile([P, LQ], F32, tag="sc", bufs=2,
                                      name=f"bc{p}{h}")
                    for n in range(2):
                        nc.tensor.matmul(
                            bc[0:D, n * 512:(n + 1) * 512],
                            ones64[:],
                            rsr_pad[:, n * 512:(n + 1) * 512],
                            start=True,
                            stop=True,
                        )
                    bcs = work.tile([D, LQ], F32, tag="bcs", bufs=2,
                                    name=f"bcs{p}{h}")
                    nc.scalar.copy(bcs[:], bc[0:D, :])
                    nc.vector.tensor_tensor(
                        attnT[base:base + D, p, :],
                        src[D:2 * D, :],
                        bcs[:],
                        mybir.AluOpType.mult,
                    )
                return emit

            with tc.tile_pool(name="work", bufs=4) as work, \
                 tc.tile_pool(name="psumSC", bufs=1, space="PSUM") as psumSC:
                # ---------------- Phase A + pass-0 scores ----------------
                ex0 = []
                with (
                    tc.tile_pool(name="inp", bufs=1) as inp,
                    tc.tile_pool(name="psumA", bufs=1, space="PSUM") as psumA,
                ):
                    wq_s = inp.tile([P, ET, FP], BF16, name="wq_s")
                    wk_s = inp.tile([P, ET, FP], BF16, name="wk_s")
                    wv_s = inp.tile([P, ET, FP], BF16, name="wv_s")
                    xT_s = [inp.tile([P, 4, LQ], BF16, tag=f"xT{j}",
                                     name=f"xT{j}") for j in range(2)]
                    cT_s = [inp.tile([P, 2, LKV], BF16, tag=f"cT{j}",
                                     name=f"cT{j}") for j in range(4)]

                    def x_chunk(k):
                        return xT_s[k // 4][:, k % 4, :]

                    def c_chunk(k):
                        return cT_s[k // 2][:, k % 2, :]

                    # Primary input stream, one queue (sync), dependency
                    # order: K inputs first (K gates pass 0), then Q's.
                    nc.sync.dma_start(
                        wk_s[:], wkT.rearrange("(ko pi) f -> pi ko f", pi=P)
                    )
                    for j in range(4):
                        nc.sync.dma_start(
                            cT_s[j][:],
                            ctxT[j * 256:(j + 1) * 256, :].rearrange(
                                "(ko pi) k -> pi ko k", pi=P
                            ),
                        )
                    nc.sync.dma_start(
                        wq_s[:], wqT.rearrange("(ko pi) f -> pi ko f", pi=P)
                    )
                    for j in range(2):
                        nc.sync.dma_start(
                            xT_s[j][:],
                            xT[j * 512:(j + 1) * 512, :].rearrange(
                                "(ko pi) q -> pi ko q", pi=P
                            ),
                        )

                    # Tail of the same stream: wv, mask quarters, fcw land
                    # in need order right behind the projection inputs.
                    nc.sync.dma_start(
                        wv_s[:], wvT.rearrange("(ko pi) f -> pi ko f", pi=P)
                    )
                    for j in range(4):
                        nc.sync.dma_start(
                            mT_all[:, j * 4 * LQ:(j + 1) * 4 * LQ].rearrange(
                                "p (kt q) -> p kt q", q=LQ
                            ),
                            maskT[j * 512:(j + 1) * 512, :].rearrange(
                                "(kt pi) q -> pi kt q", pi=P
                            ),
                        )
                    nc.sync.dma_start(
                        fcw_s[:], fcwT.rearrange("(ko pi) e -> pi ko e", pi=P)
                    )

                    # K pair 0, chunk-major across two [P, LQ] tiles borrowed
                    # from the sc tag (psumSC is otherwise idle in phase A):
                    # all output tiles accumulate chunk-by-chunk as ctx lands.
                    kp0 = [
                        psumSC.tile([P, LQ], F32, tag="sc", bufs=2,
                                    name=f"kp0_{n}")
                        for n in range(2)
                    ]
                    for k in range(ET):
                        for n in range(4):
                            nc.tensor.matmul(
                                kp0[n // 2][:, (n % 2) * 512:(n % 2 + 1) * 512],
                                wk_s[:, k, 0:P],
                                c_chunk(k)[:, n * 512:(n + 1) * 512],
                                start=(k == 0),
                                stop=(k == ET - 1),
                            )
                    for n in range(2):
                        nc.scalar.mul(
                            KTt[:, 0, n * 1024:(n + 1) * 1024], kp0[n][:], A16
                        )

                    # Q^T, both pairs, chunk-major (needs only x + wq).
                    qp = [
                        psumA.tile([P, LQ], F32, tag="ps1k", bufs=2,
                                   name=f"qp_{p}")
                        for p in range(2)
                    ]
                    for k in range(ET):
                        for p in range(2):
                            for n in range(2):
                                nc.tensor.matmul(
                                    qp[p][:, n * 512:(n + 1) * 512],
                                    wq_s[:, k, p * P:(p + 1) * P],
                                    x_chunk(k)[:, n * 512:(n + 1) * 512],
                                    start=(k == 0),
                                    stop=(k == ET - 1),
                                )
                    for p in range(2):
                        nc.vector.tensor_copy(QT[:, p, :], qp[p][:])

                    # Pass-0 scores + V projection + K pair 1, interleaved.
                    # V packs 4 LKV tiles side by side into one [P, LQ] psum.
                    def k_p1(n):
                        ps = psumA.tile([P, LQ], F32, tag="ps1k", bufs=2)
                        for k in range(ET):
                            for m in range(2):
                                nc.tensor.matmul(
                                    ps[:, m * 512:(m + 1) * 512],
                                    wk_s[:, k, P:2 * P],
                                    c_chunk(k)[:, n * 1024 + m * 512:
                                               n * 1024 + (m + 1) * 512],
                                    start=(k == 0),
                                    stop=(k == ET - 1),
                                )
                        nc.scalar.mul(
                            KTt[:, 1, n * 1024:(n + 1) * 1024], ps[:], A16
                        )

                    for g in range(4):
                        vt = psumA.tile([P, LQ], F32, tag="ps1k", bufs=2,
                                        name=f"vt{g}")
                        for j in range(4):
                            mv = 4 * g + j
                            ex0.append(emit_scores(mv, 0, 0))
                            for k in range(ET):
                                nc.tensor.matmul(
                                    vt[:, j * FP:(j + 1) * FP],
                                    c_chunk(k)[:, mv * P:(mv + 1) * P],
                                    wv_s[:, k, :],
                                    start=(k == 0),
                                    stop=(k == ET - 1),
                                )
                            nc.vector.tensor_copy(
                                Vaug[:, mv, :, D:2 * D],
                                vt[:, j * FP:(j + 1) * FP].rearrange(
                                    "p (h d) -> p h d", d=D
                                ),
                            )
                        if g % 2 == 1:
                            k_p1(g // 2)

                # ---------------- Phase B: attention ----------------
                norm_pending = None
                av3_s = None
                with tc.tile_pool(name="psumAV", bufs=1,
                                  space="PSUM") as psumAV:
                    for pi in range(4):
                        p, h = divmod(pi, 2)
                        hh = 2 * p + h
                        av = psumAV.tile([P, LQ], F32, tag="av", bufs=2,
                                         name=f"av{hh}")

                        def attv(okt, oex, pos, av=av, hh=hh):
                            for n in range(2):
                                nc.tensor.matmul(
                                    av[:, n * 512:(n + 1) * 512],
                                    Vaug[:, okt, hh, :],
                                    oex[:, n * 512:(n + 1) * 512],
                                    start=(pos == 0),
                                    stop=(pos == KT - 1),
                                )

                        if pi == 0:
                            for kt in range(KT):
                                attv(kt, ex0[kt], kt)
                        else:
                            exs = {}
                            pos = 0
                            for i, kt in enumerate(range(KT)):
                                exs[kt] = emit_scores(kt, p, h)
                                if i == 3 and norm_pending is not None:
                                    norm_pending()
                                    norm_pending = None
                                if i >= PIPE:
                                    okt = ATTV_ORDER[i - PIPE]
                                    attv(okt, exs[okt], pos)
                                    pos += 1
                            for j in range(KT - PIPE, KT):
                                okt = ATTV_ORDER[j]
                                attv(okt, exs[okt], pos)
                                pos += 1

                        if pi < 3:
                            norm_pending = make_norm(av, p, h, psumSC)
                        else:
                            # normalize in place while av is still PSUM (a
                            # tensor_tensor with both inputs in SBUF would
                            # need equal base partitions).
                            make_norm(av, p, h, psumSC)()

                # ---------------- Phase C: output projection ----------------
                with (
                    tc.tile_pool(name="psumC", bufs=1, space="PSUM") as psumC,
                    tc.tile_pool(name="outp", bufs=4) as outp,
                ):
                    CCH = 8
                    ps_c = [None] * CCH

                    def fc_mm(c, kf):
                        if kf == 0:
                            ps_c[c] = psumC.tile([P, LQ], F32, tag="fc",
                                                 bufs=2, name=f"fc{c}")
                        for n in range(2):
                            nc.tensor.matmul(
                                ps_c[c][:, n * 512:(n + 1) * 512],
                                fcw_s[:, kf, c * P:(c + 1) * P],
                                attnT[:, kf, n * 512:(n + 1) * 512],
                                start=(kf == 0),
                                stop=(kf == 1),
                            )

                    for c in range(2):
                        fc_mm(c, 0)
                    for c in range(CCH):
                        fc_mm(c, 1)
                        ob = outp.tile([P, LQ], BF16, tag="ob", bufs=4,
                                       name=f"ob{c}")
                        if c % 2 == 0:
                            nc.scalar.copy(ob[:], ps_c[c][:])
                        else:
                            nc.vector.tensor_copy(ob[:], ps_c[c][:])
                        eng = nc.sync if c % 2 == 0 else nc.gpsimd
                        eng.dma_start(yT[c * P:(c + 1) * P, :], ob[:])
                        if c + 2 < CCH:
                            fc_mm(c + 2, 0)
                    if debug:
                        nc.sync.dma_start(
                            dQT[:, :], QT.rearrange("p a q -> p (a q)"))
                        nc.sync.dma_start(
                            dKT[:, :], KTt.rearrange("p a q -> p (a q)"))
                        nc.sync.dma_start(
                            dV[:, :], Vaug.rearrange("p a b c -> p (a b c)"))
                        nc.sync.dma_start(
                            dAT[:, :], attnT.rearrange("p a q -> p (a q)"))

    nc.compile()
    return nc


_NC_CACHE: dict = {}


def _get_nc() -> bass.Bass:
    if "nc" not in _NC_CACHE:
        _NC_CACHE["nc"] = build_nc()
    return _NC_CACHE["nc"]


def make_in_maps(x, context, pad_mask, Wq, Wk, Wv, fc_w):
    x = np.asarray(x, dtype=np.float32)
    context = np.asarray(context, dtype=np.float32)
    pad_mask = np.asarray(pad_mask).astype(bool)
    Wq = np.asarray(Wq, dtype=np.float32)
    Wk = np.asarray(Wk, dtype=np.float32)
    Wv = np.asarray(Wv, dtype=np.float32)
    fc_w = np.asarray(fc_w, dtype=np.float32)

    xT = np.ascontiguousarray(x.transpose(0, 2, 1)).astype(ml_dtypes.bfloat16)
    cT = np.ascontiguousarray(context.transpose(0, 2, 1)).astype(ml_dtypes.bfloat16)
    keepT = np.ascontiguousarray(
        (~pad_mask).transpose(0, 2, 1)
    ).astype(ml_dtypes.bfloat16)                                    # [B, LKV, LQ]

    in_maps = []
    for c in range(NCORES):
        b, hg = divmod(c, HGROUPS)
        fsl = slice(hg * FP, (hg + 1) * FP)
        in_maps.append(
            {
                "xT": xT[b],
                "ctxT": cT[b],
                "maskT": keepT[b],
                "wqT": np.ascontiguousarray(Wq[fsl, :].T).astype(ml_dtypes.bfloat16),
                "wkT": np.ascontiguousarray(Wk[fsl, :].T).astype(ml_dtypes.bfloat16),
                "wvT": np.ascontiguousarray(Wv[fsl, :].T).astype(ml_dtypes.bfloat16),
                "fcwT": np.ascontiguousarray(fc_w[:, fsl].T).astype(ml_dtypes.bfloat16),
            }
        )
    return in_maps


def _combine(outs, fc_b):
    fc_b = np.asarray(fc_b, dtype=np.float32)
    y = np.empty((B, LQ, E), dtype=np.float32)
    for b in range(B):
        acc = outs[HGROUPS * b].astype(np.float32)
        for g in range(1, HGROUPS):
            acc = acc + outs[HGROUPS * b + g].astype(np.float32)
        y[b] = acc.T + fc_b
    return y


def run_traced(x, context, pad_mask, Wq, Wk, Wv, fc_w, fc_b, trace=False):
    nc = _get_nc()
    in_maps = make_in_maps(x, context, pad_mask, Wq, Wk, Wv, fc_w)
    res = run_bass_kernel_spmd(nc, in_maps, list(range(NCORES)), trace=trace)
    outs = [r["yT"] for r in res.results]
    return _combine(outs, fc_b), res


def kernel(x, context, pad_mask, Wq, Wk, Wv, fc_w, fc_b):
    y, _ = run_traced(x, context, pad_mask, Wq, Wk, Wv, fc_w, fc_b, trace=False)
    return y


# revision 22
# speedup vs baseline: 1.2589x; 1.1747x over previous
"""Cross multi-head attention kernel for 8 Trainium2 NeuronCores.

Reference computation (per batch b):
    Q = x @ Wq.T ; K = ctx @ Wk.T ; V = ctx @ Wv.T          (16 heads, depth 64)
    scores = (Q_h @ K_h.T) / 8 ; masked where pad_mask -> -inf
    att = softmax(scores) ; out_h = att @ V_h
    y = concat_h(out_h) @ fc_w.T + fc_b
Sharding: 8 cores = 2 batches x 4 head-groups (4 heads each).  Each core
computes a full [E, LQ] bf16 partial of y^T for its batch; the host sums the 4
head-group partials per batch (fp32) and adds the bias.

On-chip layout is fully transposed ("layout B") so no transposes are needed:
    x^T [E, LQ], ctx^T [E, LKV]  ->  Q^T [D,LQ], K^T [D,LKV] per head, V
    natural [LKV, D] augmented with a ones column (att@V emits softmax row
    sums for free on row 64 of the [65, LQ] accumulator).
    scores^T [LKV, LQ] = K^T.T @ Q^T       (contraction over D=64)
    att^T = exp(scores^T/8) * keep_mask^T  (exact-zero masking; no row-max
        needed: scores/8 ~ N(0,1), exp never overflows)
    y^T partial [E, LQ] = fcw_part^T.T @ attn^T   (contraction over 256)

Schedule notes (perfetto-driven; 194us baseline -> 160us):
  * The old 4 per-head passes were latency-bound (~28us for ~14us of
    work): the sc-PSUM ring round trip (score matmul -> ACT exp -> ring
    slot free) starved the PE and the HAM clock gate halved the PE clock
    for ~70us.  Phase B is now TWO merged passes (one per head pair): 32
    interleaved (kt, head) score chains per pass, so one chain's exp/mask
    latency hides behind the sibling head's matmuls; att@V runs PIPE=11
    chains behind, GpSimd-masked chains deferred to the tail (head-0
    units first so its norm overlaps head 1's tail).
  * Per merged pass the elementwise work is spread across three engines:
    22 ACT exps (scale=ln2/128 on the A16-prescaled scores); 12 masks on
    GpSimd (~2.1us each); 10 chains use a fused Schraudolph exp+mask in
    ONE DVE scalar_tensor_tensor: (A16*s + B16) * keep -> int16 whose bit
    pattern IS bf16 exp(s/8) (A16=2^7/(8 ln2) folded into the K^T evac;
    ~+-3.7%% on those att weights, self-normalizing via the row sums ->
    <0.2%% on the output; masked entries multiply to int16 0 = +0.0).
  * Softmax: att@V stationary is V augmented to 128 columns (ones at col
    0, zeros 1:63, V at 64:127), so the accumulator's row 0 is the row
    sum ON PARTITION 0 -- the custom-DVE reciprocal_approx_fast is only
    correct at base partition 0, and 64-partition-wide reads must be
    64-aligned (hence V rows at 64:127).  recip -> ones64 outer-product
    partition broadcast -> one fused multiply, done in place while av is
    in PSUM (a two-SBUF-input tensor_tensor needs equal base partitions).
    Pair-0 norms are deferred into pair-1's early chains.
  * DMA: 13 batched issues on one sync-queue stream in dependency order
    (wk, ctx x4, wq, x x2, wv, mask x4, fcw).  K pair 0 accumulates
    chunk-major into sc-tag PSUM tiles as ctx quarters land; K pair 1
    fills the ctx-done -> x-arrival window; Q is chunk-major on x; V
    (packed 4 LKV tiles per [P, LQ] psum) overlaps the first
    PRECHAINS=10 pair-0 chains inside phase A.
  * PSUM: sc 2x2 banks + av 2x2 = 8; phase A time-shares the sc tag
    (K p0) and the ps1k tag (K p1 / Q / V) before psumAV opens.
    TRN2 constraints hit on the way: matmul PSUM writes max 512 fp32
    wide (one bank); partition reads 32/64-aligned at 32/64-wide.
"""

import os
import sys

import numpy as np

for _p in ("/opt/trn_rl_repo", "/root/.axon_site/_ro/trn_rl_repo"):
    if os.path.isdir(_p) and _p not in sys.path:
        sys.path.insert(0, _p)

import ml_dtypes  # noqa: E402

import concourse.bass as bass  # noqa: E402
import concourse.mybir as mybir  # noqa: E402
import concourse.tile as tile  # noqa: E402
from concourse import bacc  # noqa: E402
from concourse.bass_utils import run_bass_kernel_spmd  # noqa: E402

B, LQ, LKV, E = 2, 1024, 2048, 1024
H_TOTAL, D = 16, 64
NCORES = 8
HGROUPS = 4          # head groups (cores per batch)
HLOCAL = 4           # heads per core
FP = HLOCAL * D      # 256 local head features
P = 128
F32 = mybir.dt.float32
BF16 = mybir.dt.bfloat16
I16 = mybir.dt.int16
ET = E // P          # 8 contraction tiles for the projections
KT = LKV // P        # 16 key tiles
PIPE = 6             # att@V runs this many kt tiles behind the scores

# Schraudolph constants: bf16 bit pattern i = 2^7*(127 + log2(e)*s/8 - delta)
A16 = 128.0 / (8.0 * float(np.log(2.0)))        # K^T pre-scale: 23.0831
B16 = 16256.0 - 128.0 * 0.0354 + 0.25           # +0.25 hedges round-vs-trunc
EXP_SCALE = float(np.log(2.0)) / 128.0          # ACT exp scale on A16*s

GP_SET = (0, 4, 8, 12)  # ACT exp, GpSimd mask       # ACT exp, GpSimd mask (att@V deferred to end)
STT_SET = (3, 7, 11, 15)   # fused DVE Schraudolph exp+mask     # fused DVE Schraudolph exp+mask
ATTV_ORDER = [kt for kt in range(KT) if kt not in GP_SET] + list(GP_SET)


def build_nc(debug: bool = False) -> bass.Bass:
    nc = bacc.Bacc("TRN2", target_bir_lowering=False)

    xT = nc.dram_tensor("xT", [E, LQ], BF16, kind="ExternalInput")
    ctxT = nc.dram_tensor("ctxT", [E, LKV], BF16, kind="ExternalInput")
    maskT = nc.dram_tensor("maskT", [LKV, LQ], BF16, kind="ExternalInput")
    wqT = nc.dram_tensor("wqT", [E, FP], BF16, kind="ExternalInput")
    wkT = nc.dram_tensor("wkT", [E, FP], BF16, kind="ExternalInput")
    wvT = nc.dram_tensor("wvT", [E, FP], BF16, kind="ExternalInput")
    fcwT = nc.dram_tensor("fcwT", [FP, E], BF16, kind="ExternalInput")
    yT = nc.dram_tensor("yT", [E, LQ], BF16, kind="ExternalOutput")
    if debug:
        dQT = nc.dram_tensor("dQT", [P, 2 * LQ], BF16, kind="ExternalOutput")
        dKT = nc.dram_tensor("dKT", [P, 2 * LKV], BF16, kind="ExternalOutput")
        dV = nc.dram_tensor("dV", [P, KT * HLOCAL * (D + 1)], BF16,
                            kind="ExternalOutput")
        dAT = nc.dram_tensor("dAT", [P, 2 * LQ], BF16, kind="ExternalOutput")

    with tile.TileContext(nc) as tc:
        with tc.tile_pool(name="persist", bufs=1) as persist:
            QT = persist.tile([P, 2, LQ], BF16)        # [:, pair, :]
            KTt = persist.tile([P, 2, LKV], BF16)      # pre-scaled by A16
            Vaug = persist.tile([P, KT, HLOCAL, P], BF16)
            attnT = persist.tile([P, 2, LQ], BF16)
            fcw_s = persist.tile([P, 2, E], BF16)
            mT_all = persist.tile([P, KT * LQ], BF16, name="mT_all")
            # zero-padded broadcast operands: row 0 live, rows 1-127 zero so
            # the K=128 outer-product matmul is exact.
            ones64 = persist.tile([P, D], F32)
            rsr_pad = persist.tile([P, LQ], F32)

            # Preload an exp-capable table set during the DMA shadow.
            nc.scalar.add_instruction(
                mybir.InstLoadActFuncSet(
                    name=nc.scalar.bass.get_next_instruction_name(),
                    act_func_set_id=6,  # natural_log_exp_and_others
                    ins=[],
                    outs=[],
                )
            )
            nc.vector.memset(ones64[:], 0.0)
            nc.vector.memset(ones64[0:1, :], 1.0)
            nc.vector.memset(rsr_pad[:], 0.0)
            # col 0 = ones (row sums land on partition 0 of the av psum,
            # where the custom-DVE recip is valid); cols 1:64 = zeros (64-wide
            # partition reads must start 64-aligned, so V rows live at 64:128).
            nc.gpsimd.memset(Vaug[:, :, :, 0:1], 1.0)
            nc.gpsimd.memset(Vaug[:, :, :, 1:64], 0.0)

            def mask_ap(kt):
                return mT_all[:, kt * LQ:(kt + 1) * LQ]

            work = None
            psumSC = None

            def emit_scores(kt, p, h, c):
                base = h * D
                sc = psumSC.tile([P, LQ], F32, tag="sc", bufs=2,
                                 name=f"sc_{p}{h}{kt}")
                for n in range(2):
                    nc.tensor.matmul(
                        sc[:, n * 512:(n + 1) * 512],
                        KTt[base:base + D, p, kt * P:(kt + 1) * P],
                        QT[base:base + D, p, n * 512:(n + 1) * 512],
                        start=True,
                        stop=True,
                    )
                ex = work.tile([P, LQ], BF16, tag="ex", bufs=26,
                               name=f"ex_{p}{h}{kt}")
                if c in STT_SET:
                    # i16 = (A16*s + B16) * keep; bit pattern == bf16 exp(s/8)
                    nc.vector.scalar_tensor_tensor(
                        ex[:].bitcast(I16),
                        sc[:],
                        B16,
                        mask_ap(kt),
                        mybir.AluOpType.add,
                        mybir.AluOpType.mult,
                    )
                else:
                    nc.scalar.activation(
                        ex[:], sc[:], mybir.ActivationFunctionType.Exp,
                        scale=EXP_SCALE,
                    )
                    eng = nc.gpsimd if c in GP_SET else nc.vector
                    eng.tensor_tensor(
                        ex[:], ex[:], mask_ap(kt), mybir.AluOpType.mult
                    )
                return ex

            def make_norm(src, p, h, bc_pool):
                # src: [D+1, LQ] accumulator (PSUM or SBUF fp32): rows 0:D are
                # unnormalized att@V, row D the softmax row-sum.
                def emit():
                    base = h * D
                    nc.vector.reciprocal_approx_fast(
                        rsr_pad[0:1, :], src[0:1, :]
                    )
                    bc = bc_pool.tile([P, LQ], F32, tag="sc", bufs=2,
                                      name=f"bc{p}{h}")
                    for n in range(2):
                        nc.tensor.matmul(
                            bc[0:D, n * 512:(n + 1) * 512],
                            ones64[:],
                            rsr_pad[:, n * 512:(n + 1) * 512],
                            start=True,
                            stop=True,
                        )
                    bcs = work.tile([D, LQ], F32, tag="bcs", bufs=2,
                                    name=f"bcs{p}{h}")
                    nc.scalar.copy(bcs[:], bc[0:D, :])
                    nc.vector.tensor_tensor(
                        attnT[base:base + D, p, :],
                        src[D:2 * D, :],
                        bcs[:],
                        mybir.AluOpType.mult,
                    )
                return emit

            with tc.tile_pool(name="work", bufs=4) as work, \
                 tc.tile_pool(name="psumSC", bufs=1, space="PSUM") as psumSC:
                # ---------------- Phase A + pass-0 scores ----------------
                exs0: dict = {}
                with (
                    tc.tile_pool(name="inp", bufs=1) as inp,
                    tc.tile_pool(name="psumA", bufs=1, space="PSUM") as psumA,
                ):
                    wq_s = inp.tile([P, ET, FP], BF16, name="wq_s")
                    wk_s = inp.tile([P, ET, FP], BF16, name="wk_s")
                    wv_s = inp.tile([P, ET, FP], BF16, name="wv_s")
                    xT_s = [inp.tile([P, 4, LQ], BF16, tag=f"xT{j}",
                                     name=f"xT{j}") for j in range(2)]
                    cT_s = [inp.tile([P, 2, LKV], BF16, tag=f"cT{j}",
                                     name=f"cT{j}") for j in range(4)]

                    def x_chunk(k):
                        return xT_s[k // 4][:, k % 4, :]

                    def c_chunk(k):
                        return cT_s[k // 2][:, k % 2, :]

                    # Primary input stream, one queue (sync), dependency
                    # order: K inputs first (K gates pass 0), then Q's.
                    nc.sync.dma_start(
                        wk_s[:], wkT.rearrange("(ko pi) f -> pi ko f", pi=P)
                    )
                    for j in range(4):
                        nc.sync.dma_start(
                            cT_s[j][:],
                            ctxT[j * 256:(j + 1) * 256, :].rearrange(
                                "(ko pi) k -> pi ko k", pi=P
                            ),
                        )
                    nc.sync.dma_start(
                        wq_s[:], wqT.rearrange("(ko pi) f -> pi ko f", pi=P)
                    )
                    for j in range(2):
                        nc.sync.dma_start(
                            xT_s[j][:],
                            xT[j * 512:(j + 1) * 512, :].rearrange(
                                "(ko pi) q -> pi ko q", pi=P
                            ),
                        )

                    # Tail of the same stream: wv, mask quarters, fcw land
                    # in need order right behind the projection inputs.
                    nc.sync.dma_start(
                        wv_s[:], wvT.rearrange("(ko pi) f -> pi ko f", pi=P)
                    )
                    for j in range(4):
                        nc.sync.dma_start(
                            mT_all[:, j * 4 * LQ:(j + 1) * 4 * LQ].rearrange(
                                "p (kt q) -> p kt q", q=LQ
                            ),
                            maskT[j * 512:(j + 1) * 512, :].rearrange(
                                "(kt pi) q -> pi kt q", pi=P
                            ),
                        )
                    nc.sync.dma_start(
                        fcw_s[:], fcwT.rearrange("(ko pi) e -> pi ko e", pi=P)
                    )

                    # K pair 0, chunk-major across two [P, LQ] tiles borrowed
                    # from the sc tag (psumSC is otherwise idle in phase A):
                    # all output tiles accumulate chunk-by-chunk as ctx lands.
                    kp0 = [
                        psumSC.tile([P, LQ], F32, tag="sc", bufs=2,
                                    name=f"kp0_{n}")
                        for n in range(2)
                    ]
                    for k in range(ET):
                        for n in range(4):
                            nc.tensor.matmul(
                                kp0[n // 2][:, (n % 2) * 512:(n % 2 + 1) * 512],
                                wk_s[:, k, 0:P],
                                c_chunk(k)[:, n * 512:(n + 1) * 512],
                                start=(k == 0),
                                stop=(k == ET - 1),
                            )
                    for n in range(2):
                        nc.scalar.mul(
                            KTt[:, 0, n * 1024:(n + 1) * 1024], kp0[n][:], A16
                        )

                    # K pair 1 fills the ctx-done -> x-arrival window.
                    kp1 = [
                        psumA.tile([P, LQ], F32, tag="ps1k", bufs=2,
                                   name=f"kp1_{n}")
                        for n in range(2)
                    ]
                    for k in range(ET):
                        for n in range(4):
                            nc.tensor.matmul(
                                kp1[n // 2][:, (n % 2) * 512:(n % 2 + 1) * 512],
                                wk_s[:, k, P:2 * P],
                                c_chunk(k)[:, n * 512:(n + 1) * 512],
                                start=(k == 0),
                                stop=(k == ET - 1),
                            )
                    for n in range(2):
                        nc.scalar.mul(
                            KTt[:, 1, n * 1024:(n + 1) * 1024], kp1[n][:], A16
                        )

                    # Q^T, both pairs, chunk-major (needs only x + wq).
                    qp = [
                        psumA.tile([P, LQ], F32, tag="ps1k", bufs=2,
                                   name=f"qp_{p}")
                        for p in range(2)
                    ]
                    for k in range(ET):
                        for p in range(2):
                            for n in range(2):
                                nc.tensor.matmul(
                                    qp[p][:, n * 512:(n + 1) * 512],
                                    wq_s[:, k, p * P:(p + 1) * P],
                                    x_chunk(k)[:, n * 512:(n + 1) * 512],
                                    start=(k == 0),
                                    stop=(k == ET - 1),
                                )
                    for p in range(2):
                        nc.vector.tensor_copy(QT[:, p, :], qp[p][:])

                    # V projection + the first PRECHAINS pair-0 score
                    # chains interleaved (chain c = 2*kt + h).  V packs 4
                    # LKV tiles side by side into one [P, LQ] psum.
                    for g in range(4):
                        vt = psumA.tile([P, LQ], F32, tag="ps1k", bufs=2,
                                        name=f"vt{g}")
                        for j in range(4):
                            mv = 4 * g + j
                            if 2 * mv < PRECHAINS:
                                for h in range(2):
                                    c = 2 * mv + h
                                    exs0[c] = emit_scores(mv, 0, h, c)
                            for k in range(ET):
                                nc.tensor.matmul(
                                    vt[:, j * FP:(j + 1) * FP],
                                    c_chunk(k)[:, mv * P:(mv + 1) * P],
                                    wv_s[:, k, :],
                                    start=(k == 0),
                                    stop=(k == ET - 1),
                                )
                            nc.vector.tensor_copy(
                                Vaug[:, mv, :, D:2 * D],
                                vt[:, j * FP:(j + 1) * FP].rearrange(
                                    "p (h d) -> p h d", d=D
                                ),
                            )

                # ---------------- Phase B: attention ----------------
                # Two merged passes, one per head pair: 32 interleaved score
                # chains (kt, h) so one chain's exp/mask latency hides behind
                # the sibling head's matmuls; att@V runs PIPE chains behind,
                # GpSimd-masked chains deferred to the tail.  Both heads'
                # softmax norms run at pass end while av is still in PSUM.
                with tc.tile_pool(name="psumAV", bufs=1,
                                  space="PSUM") as psumAV:
                    pending_norms = []
                    for pair in range(2):
                        avs = [
                            psumAV.tile([P, LQ], F32, tag="av", bufs=2,
                                        name=f"av{pair}{h}")
                            for h in range(2)
                        ]
                        poscnt = [0, 0]
                        exs = exs0 if pair == 0 else {}

                        def attv_c(c, pair=pair, avs=avs, poscnt=poscnt,
                                   exs=exs):
                            kt, h = divmod(c, 2)
                            hh = 2 * pair + h
                            pos = poscnt[h]
                            for n in range(2):
                                nc.tensor.matmul(
                                    avs[h][:, n * 512:(n + 1) * 512],
                                    Vaug[:, kt, hh, :],
                                    exs[c][:, n * 512:(n + 1) * 512],
                                    start=(pos == 0),
                                    stop=(pos == KT - 1),
                                )
                            poscnt[h] = pos + 1

                        first = PRECHAINS if pair == 0 else 0
                        for i in range(first, CHAINS):
                            kt, h = divmod(i, 2)
                            exs[i] = emit_scores(kt, pair, h, i)
                            if i == 5 and pending_norms:
                                # previous pair's softmax norms, hidden under
                                # this pair's early score chains
                                for f in pending_norms:
                                    f()
                                pending_norms = []
                            if i >= PIPE:
                                attv_c(ATTV_ORDER[i - PIPE])
                        for q in range(CHAINS - PIPE, CHAINS):
                            attv_c(ATTV_ORDER[q])
                        # norms in place while av is still PSUM (a tensor_tensor
                        # with two SBUF inputs needs equal base partitions)
                        if pair == 0:
                            pending_norms = [
                                make_norm(avs[h], pair, h, psumSC)
                                for h in range(2)
                            ]
                        else:
                            for h in range(2):
                                make_norm(avs[h], pair, h, psumSC)()

                # ---------------- Phase C: output projection ----------------
                with (
                    tc.tile_pool(name="psumC", bufs=1, space="PSUM") as psumC,
                    tc.tile_pool(name="outp", bufs=4) as outp,
                ):
                    CCH = 8
                    ps_c = [None] * CCH

                    def fc_mm(c, kf):
                        if kf == 0:
                            ps_c[c] = psumC.tile([P, LQ], F32, tag="fc",
                                                 bufs=2, name=f"fc{c}")
                        for n in range(2):
                            nc.tensor.matmul(
                                ps_c[c][:, n * 512:(n + 1) * 512],
                                fcw_s[:, kf, c * P:(c + 1) * P],
                                attnT[:, kf, n * 512:(n + 1) * 512],
                                start=(kf == 0),
                                stop=(kf == 1),
                            )

                    for c in range(2):
                        fc_mm(c, 0)
                    for c in range(CCH):
                        fc_mm(c, 1)
                        ob = outp.tile([P, LQ], BF16, tag="ob", bufs=4,
                                       name=f"ob{c}")
                        if c % 2 == 0:
                            nc.scalar.copy(ob[:], ps_c[c][:])
                        else:
                            nc.vector.tensor_copy(ob[:], ps_c[c][:])
                        eng = nc.sync if c % 2 == 0 else nc.gpsimd
                        eng.dma_start(yT[c * P:(c + 1) * P, :], ob[:])
                        if c + 2 < CCH:
                            fc_mm(c + 2, 0)
                    if debug:
                        nc.sync.dma_start(
                            dQT[:, :], QT.rearrange("p a q -> p (a q)"))
                        nc.sync.dma_start(
                            dKT[:, :], KTt.rearrange("p a q -> p (a q)"))
                        nc.sync.dma_start(
                            dV[:, :], Vaug.rearrange("p a b c -> p (a b c)"))
                        nc.sync.dma_start(
                            dAT[:, :], attnT.rearrange("p a q -> p (a q)"))

    nc.compile()
    return nc


_NC_CACHE: dict = {}


def _get_nc() -> bass.Bass:
    if "nc" not in _NC_CACHE:
        _NC_CACHE["nc"] = build_nc()
    return _NC_CACHE["nc"]


def make_in_maps(x, context, pad_mask, Wq, Wk, Wv, fc_w):
    x = np.asarray(x, dtype=np.float32)
    context = np.asarray(context, dtype=np.float32)
    pad_mask = np.asarray(pad_mask).astype(bool)
    Wq = np.asarray(Wq, dtype=np.float32)
    Wk = np.asarray(Wk, dtype=np.float32)
    Wv = np.asarray(Wv, dtype=np.float32)
    fc_w = np.asarray(fc_w, dtype=np.float32)

    xT = np.ascontiguousarray(x.transpose(0, 2, 1)).astype(ml_dtypes.bfloat16)
    cT = np.ascontiguousarray(context.transpose(0, 2, 1)).astype(ml_dtypes.bfloat16)
    keepT = np.ascontiguousarray(
        (~pad_mask).transpose(0, 2, 1)
    ).astype(ml_dtypes.bfloat16)                                    # [B, LKV, LQ]

    in_maps = []
    for c in range(NCORES):
        b, hg = divmod(c, HGROUPS)
        fsl = slice(hg * FP, (hg + 1) * FP)
        in_maps.append(
            {
                "xT": xT[b],
                "ctxT": cT[b],
                "maskT": keepT[b],
                "wqT": np.ascontiguousarray(Wq[fsl, :].T).astype(ml_dtypes.bfloat16),
                "wkT": np.ascontiguousarray(Wk[fsl, :].T).astype(ml_dtypes.bfloat16),
                "wvT": np.ascontiguousarray(Wv[fsl, :].T).astype(ml_dtypes.bfloat16),
                "fcwT": np.ascontiguousarray(fc_w[:, fsl].T).astype(ml_dtypes.bfloat16),
            }
        )
    return in_maps


def _combine(outs, fc_b):
    fc_b = np.asarray(fc_b, dtype=np.float32)
    y = np.empty((B, LQ, E), dtype=np.float32)
    for b in range(B):
        acc = outs[HGROUPS * b].astype(np.float32)
        for g in range(1, HGROUPS):
            acc = acc + outs[HGROUPS * b + g].astype(np.float32)
        y[b] = acc.T + fc_b
    return y


def run_traced(x, context, pad_mask, Wq, Wk, Wv, fc_w, fc_b, trace=False):
    nc = _get_nc()
    in_maps = make_in_maps(x, context, pad_mask, Wq, Wk, Wv, fc_w)
    res = run_bass_kernel_spmd(nc, in_maps, list(range(NCORES)), trace=trace)
    outs = [r["yT"] for r in res.results]
    return _combine(outs, fc_b), res


def kernel(x, context, pad_mask, Wq, Wk, Wv, fc_w, fc_b):
    y, _ = run_traced(x, context, pad_mask, Wq, Wk, Wv, fc_w, fc_b, trace=False)
    return y
